# revision 9
# baseline (speedup 1.0000x reference)
"""CapsuleConv2d Trainium2 kernel.

Math: out[b,o,h,w,i,j] = sum_{ci,kh,kw} W[j,o,ci,kh,kw] * x[b,ci,h+kh-1,w+kw-1,i,0]
i.e. a 3x3 pad-1 conv with effective batch (b,i): 64 images [64,56,56],
Cout = 256 (co = j*64+o).

Strategy (8 cores, data-parallel over b):
  - each core takes 2 of 16 b-groups; the 4 ic0 images of a b-group ride in
    the free dim (w,i) so HBM loads are fully contiguous.
  - b-group 0 lives in SBUF partitions 0-63, b-group 1 in partitions 64-127;
    each PE row-quadrant-half computes one group's full 256 output channels
    (weights duplicated across both partition halves). The two 64-row matmul
    streams run concurrently on the PE (row tiling via base_partition).
  - x is stored [ci, h_pad, (w_pad, i)] with a zero halo; each conv offset
    (kh,kw) is a sliced matmul rhs, accumulated in PSUM over 9 offsets.
  - fp16 inputs/weights (halves DMA + enables fast weight load), fp32 PSUM
    accumulate, bf16 output (halves output DMA); host converts/transposes.
"""

import sys

if "/opt/trn_rl_repo" not in sys.path:
    sys.path.insert(0, "/opt/trn_rl_repo")

import numpy as np

NCORES = 8
B, C, H, W_, IC0, WC1, O = 16, 64, 56, 56, 4, 4, 64
CO = WC1 * O  # 256
BPC = B // NCORES  # 2 b-groups per core
WI = W_ * IC0  # 224 = free-dim run per image row
HP, WP = H + 2, (W_ + 2) * IC0  # padded sbuf tile dims: 58, 232
NKER = 9

# h-tile ownership per x chunk: chunk c serves h-tiles [TS[c], TS[c+1]);
# its SBUF tile holds padded rows [2*TS[c], 2*TS[c+1]+2) (2-row halo overlap)
# so every h-tile's 4-row window lives in one tile and the first matmul only
# waits on chunk 0 (Tile deps are whole-tile). Chunk 0 is small so the
# matmul stream starts as early as possible.
TS = [0, 2, 6, 11, 17, 23, 28]
CHUNK_ROWS = [(2 * TS[c], 2 * TS[c + 1] + 2) for c in range(len(TS) - 1)]

_COMPILED = None


def _build():
    import concourse.tile as tile
    from concourse import bacc, mybir

    dt = mybir.dt
    DT = dt.float16

    nc = bacc.Bacc(
        "TRN2", target_bir_lowering=False, debug=False, num_devices=NCORES
    )
    x_d = nc.dram_tensor("x", [BPC, C, HP, WP], DT, kind="ExternalInput").ap()
    w_d = nc.dram_tensor("w", [128, NKER, CO], DT, kind="ExternalInput").ap()
    y_d = nc.dram_tensor(
        "y", [BPC, CO, H, WI], dt.bfloat16, kind="ExternalOutput"
    ).ap()

    with tile.TileContext(nc) as tc:
        with (
            tc.tile_pool(name="xp", bufs=1) as xp,
            tc.tile_pool(name="wp", bufs=1) as wp,
            tc.tile_pool(name="op", bufs=3) as op,
            tc.tile_pool(name="pp", bufs=2, space="PSUM") as pp,
        ):
            xts = []
            for c, (r0, r1) in enumerate(CHUNK_ROWS):
                xc = xp.tile([128, r1 - r0, WP], DT, name=f"x{c}")
                xts.append(xc)

            # scratch tile for HAM warmup matmuls: zeroed by DVE, so the PE
            # can start (and warm the 2.4 GHz clock gate) with no DMA dep.
            scr = wp.tile([128, 448], DT, name="scr")
            nc.vector.memset(scr[:, :], 0.0)

            # weights split by k so the first matmuls only wait on the k0-2
            # third; loaded on the scalar queue in parallel with x chunk 0 on
            # sync. All other DMAs (x chunks, y out) stay on sync — spreading
            # them across queues measurably slows the PE stream.
            wta = wp.tile([128, 3, CO], DT, name="wta")
            wtb = wp.tile([128, 6, CO], DT, name="wtb")
            for g in range(BPC):
                nc.sync.dma_start(
                    xts[0][64 * g : 64 * g + 64, :, :],
                    x_d[g, :, CHUNK_ROWS[0][0] : CHUNK_ROWS[0][1], :],
                )
            nc.scalar.dma_start(wta[:, :, :], w_d[:, 0:3, :])
            nc.scalar.dma_start(wtb[:, :, :], w_d[:, 3:9, :])
            for c in range(1, len(CHUNK_ROWS)):
                r0, r1 = CHUNK_ROWS[c]
                for g in range(BPC):
                    nc.sync.dma_start(
                        xts[c][64 * g : 64 * g + 64, :, :], x_d[g, :, r0:r1, :]
                    )

            pw = pp.tile([128, 448], dt.float32, tag="p00", name="pw")
            for _ in range(8):
                nc.tensor.matmul(
                    pw[:, :],
                    lhsT=scr[0:64, 0:128],
                    rhs=scr[0:64, 0:448],
                    start=True,
                    stop=True,
                )

            for ht in range(H // 2):
                c = next(i for i in range(len(TS) - 1) if TS[i] <= ht < TS[i + 1])
                xc = xts[c]
                lh0 = 2 * ht - CHUNK_ROWS[c][0]
                h0 = 2 * ht
                for hf in range(2):
                    ps = [
                        pp.tile(
                            [128, 2, WI], dt.float32, tag=f"p{g}{hf}", name=f"p{g}{hf}"
                        )
                        for g in range(2)
                    ]
                    for k in range(NKER):
                        kh, kw = divmod(k, 3)
                        c0 = IC0 * kw
                        wtile, kk = (wta, k) if k < 3 else (wtb, k - 3)
                        for g in range(2):
                            nc.tensor.matmul(
                                ps[g][:, :, :],
                                lhsT=wtile[
                                    64 * g : 64 * g + 64, kk, 128 * hf : 128 * hf + 128
                                ],
                                rhs=xc[
                                    64 * g : 64 * g + 64,
                                    lh0 + kh : lh0 + kh + 2,
                                    c0 : c0 + WI,
                                ],
                                start=(k == 0),
                                stop=(k == NKER - 1),
                            )
                    for g in range(2):
                        o = op.tile(
                            [128, 2, WI], dt.bfloat16, tag=f"o{g}{hf}", name=f"o{g}{hf}"
                        )
                        nc.vector.tensor_copy(o[:, :, :], ps[g][:, :, :])
                        nc.sync.dma_start(
                            y_d[g, 128 * hf : 128 * hf + 128, h0 : h0 + 2, :],
                            o[:, :, :],
                        )

    nc.compile()
    return nc


def _prep(x, W):
    xs = np.asarray(x, np.float32).reshape(B, C, H, WI)  # drop ic1, fuse (w,i)
    xpad = np.zeros((B, C, HP, WP), np.float16)  # host zero-pad = sbuf halo
    xpad[:, :, 1 : H + 1, IC0 : IC0 + WI] = xs.astype(np.float16)
    Wf = np.asarray(W, np.float32).reshape(CO, C, 3, 3)
    Wt = np.ascontiguousarray(Wf.transpose(1, 2, 3, 0)).reshape(C, NKER, CO)
    # duplicate across both partition halves: rows 0-63 serve b-group 0's
    # quadrant, rows 64-127 serve b-group 1's.
    wsb = np.ascontiguousarray(np.concatenate([Wt, Wt], axis=0).astype(np.float16))
    return xpad, wsb


def _run(x, W, trace=False):
    global _COMPILED
    from concourse.bass_utils import run_bass_kernel_spmd

    if _COMPILED is None:
        _COMPILED = _build()
    nc = _COMPILED
    xs, wsb = _prep(x, W)
    in_maps = [
        {"x": np.ascontiguousarray(xs[c * BPC : (c + 1) * BPC]), "w": wsb}
        for c in range(NCORES)
    ]
    res = run_bass_kernel_spmd(
        nc, in_maps, core_ids=list(range(NCORES)), trace=trace
    )
    ys = np.concatenate(
        [np.asarray(res.results[c]["y"], dtype=np.float32) for c in range(NCORES)],
        axis=0,
    )  # [16, 256, 56, 224]
    out = (
        ys.reshape(B, WC1, O, H, W_, IC0)
        .transpose(0, 2, 3, 4, 5, 1)
        .astype(np.float32)
    )
    return np.ascontiguousarray(out), res


def kernel(**inputs) -> np.ndarray:
    return _run(inputs["x"], inputs["W"])[0]
